# revision 34
# baseline (speedup 1.0000x reference)
"""Expert-parallel MoE kernel for Trainium2 (8 NeuronCores).

Strategy (hardcoded for B=4, S=2048, D=768, H=3072, E=8, K=2, cap_factor=1.5):
  - Host: router (x @ Wr, softmax, top-2, capacity-limited keep in flat order),
    then dispatch: gather each expert's kept tokens (<= capacity 1536) into a
    dense per-expert buffer. This is the "all-to-all dispatch" done at
    shard-time on the host.
  - Device (SPMD, one expert per core): dense fused FFN over the expert's
    token buffer: y = gelu(x @ w1 + b1) @ w2 + b2, bf16 matmuls with fp32
    accumulate, gelu/bias on the scalar engine. 13.9% of the mm1
    contraction (ko 0-1 for 20 of 24 h-chunks) runs as fp8 e4m3 DoubleRow
    matmuls (~235ns vs 2x216ns bf16, ~11us saved) — measured max rel err
    1.886e-2 vs the 2e-2 gate, numpy-predicted to 3 digits.
  - Host: combine: out[token] += combine_weight * y  (scatter-add, unshard).

Perf budget (exec window = first compute-engine slice -> last slice):
  ~6us framework preamble floor, ~7us warmup while pre0 streams (input DMA
  path caps ~260-280 GB/s early, 8-core HBM contention), ~166us dense
  matmul stream (<1us of gaps), ~11us tail (act+store+drain ~3us, codegen'd postamble
  that resets sems S[3..53] per engine ~8us — fixed, independent of queue
  declarations).

Self-contained: only needs numpy/ml_dtypes/concourse (+ axon jax devices).
"""
import os
import numpy as np
import ml_dtypes

B, S, D, H, E, TOPK = 4, 2048, 768, 3072, 8, 2
N_TOK = B * S
CAP = int((N_TOK / E) * 1.5)  # 1536
P = 128
TB = 512                      # token block (matmul free dim)
NBLK = CAP // TB              # 3
KO = D // P                   # 6  (d-chunks)
MH = H // P                   # 24 (h-chunks)
NCORES = 8

_CACHE = {}


def _ensure_ntff_hook_importable():
    """concourse.bass_utils' trace path does `from antenv.axon_hooks import
    get_axon_ntff_profile_hook`, which doesn't exist on slim axon images. If
    it's missing, register a stub so tracing degrades gracefully instead of
    crashing; when the axon .so with NRT-profile symbols is present, provide
    a working hook so NTFF profiling (HW exec time) works too."""
    import sys
    import types
    try:
        import antenv.axon_hooks  # noqa: F401
        return
    except ImportError:
        pass

    hook = None
    try:
        import contextlib
        import ctypes
        lib = ctypes.CDLL("/opt/axon/libaxon_pjrt.so")
        lib.axon_start_nrt_profile.argtypes = [
            ctypes.POINTER(ctypes.c_int64), ctypes.c_size_t]
        lib.axon_start_nrt_profile.restype = ctypes.c_int64
        lib.axon_stop_nrt_profile.argtypes = [ctypes.c_char_p]
        lib.axon_stop_nrt_profile.restype = ctypes.c_int64

        @contextlib.contextmanager
        def _hook(output_dir, device_ids):
            import jax
            jax.devices()
            if device_ids:
                ids = (ctypes.c_int64 * len(device_ids))(*device_ids)
                rc = lib.axon_start_nrt_profile(ids, len(device_ids))
            else:
                rc = lib.axon_start_nrt_profile(None, 0)
            if rc != 0:
                raise RuntimeError(f"axon_start_nrt_profile rc={rc}")
            try:
                yield
            finally:
                lib.axon_stop_nrt_profile(str(output_dir).encode())

        hook = _hook
    except Exception:
        hook = None

    mod = types.ModuleType("antenv.axon_hooks")
    mod.get_axon_ntff_profile_hook = lambda: hook
    mod.set_axon_ntff_profile_hook = lambda h: None
    sys.modules["antenv.axon_hooks"] = mod


def _build_nc():
    """Build + compile the per-core Bass program (identical on all 8 cores)."""
    from contextlib import ExitStack
    import concourse.mybir as mybir
    import concourse.tile as tile
    from concourse import bacc

    nc = bacc.Bacc("TRN2", target_bir_lowering=False, debug=False,
                   num_devices=NCORES)
    f32, bf16 = mybir.dt.float32, mybir.dt.bfloat16

    # Layouts (host pre-arranged so every DMA is contiguous):
    #  xeT[pi, blk, ko, t]  = x_e[blk*TB + t, ko*P + pi]
    #  w1 [pi, mh, ko, hi]  = w1_e[ko*P + pi, mh*P + hi]   (lhsT tiles for mm1)
    #  w2 [pi, d,  kh, di]  = w2_e[kh*P + pi, d*P + di]    (lhsT tiles for mm2)
    #  b1 [pi, mh] = b1_e[mh*P + pi],  b2[pi, d] = b2_e[d*P + pi]
    #  out[pi, blk, ko, t]  = y_e[blk*TB + t, ko*P + pi]
    #  pre0[pi, ko, 0:TB] = xe blk0; [TB:TB+P] = w1_e[ko*P+pi, 0:P] (mh=0 tiles)
    GELU = mybir.ActivationFunctionType.Gelu
    IDENT = mybir.ActivationFunctionType.Identity
    DR = mybir.MatmulPerfMode.DoubleRow
    f8e4 = mybir.dt.float8e4
    SFP8 = MH - 20  # mh >= SFP8 run ko 0-1 as one fp8 DoubleRow matmul

    pre0 = nc.dram_tensor("pre0", [P, KO, TB + P], bf16, kind="ExternalInput").ap()
    xeT = nc.dram_tensor("xeT", [P, NBLK, KO, TB], bf16, kind="ExternalInput").ap()
    # w1 ships in three pieces: full bf16 for the non-fp8 mh chunks, ko 2-5
    # bf16 for the fp8 ones, and the ko 0-1 slice as host-quantized e4m3 —
    # 590KB less on the supply-critical early DMA path than shipping full
    # bf16 + converting on device, and the w leg is single-rounded
    # (fp32->e4m3), which also improves max rel err 1.90e-2 -> 1.79e-2.
    w1a = nc.dram_tensor("w1a", [P, SFP8, KO, P], bf16, kind="ExternalInput").ap()
    w1b = nc.dram_tensor("w1b", [P, MH - SFP8, KO - 2, P], bf16,
                         kind="ExternalInput").ap()
    w1f8 = nc.dram_tensor("w1f8", [P, MH - SFP8, 2, P], f8e4,
                          kind="ExternalInput").ap()
    b1 = nc.dram_tensor("b1", [P, MH], f32, kind="ExternalInput").ap()
    w2 = nc.dram_tensor("w2", [P, KO, MH, P], bf16, kind="ExternalInput").ap()
    b2 = nc.dram_tensor("b2", [P, KO], f32, kind="ExternalInput").ap()
    out = nc.dram_tensor("out", [P, NBLK, KO, TB], f32, kind="ExternalOutput").ap()

    with tile.TileContext(nc) as tc, ExitStack() as ctx:
        consts = ctx.enter_context(tc.tile_pool(name="consts", bufs=1))
        hpool = ctx.enter_context(tc.tile_pool(name="hpool", bufs=2))
        ypool = ctx.enter_context(tc.tile_pool(name="ypool", bufs=2))
        ps1 = ctx.enter_context(tc.tile_pool(name="ps1", bufs=3, space="PSUM"))
        ps2 = ctx.enter_context(tc.tile_pool(name="ps2", bufs=2, space="PSUM"))
        pst = ctx.enter_context(tc.tile_pool(name="pst", bufs=2, space="PSUM"))
        psw = ctx.enter_context(tc.tile_pool(name="psw", bufs=1, space="PSUM"))

        # PE warm-up: cold matmuls ramp the PE clock 1.2->2.4 GHz while the
        # first input DMAs are in flight, so the real stream runs hot from
        # its first instruction. Nothing can start before ~7.2us: the
        # framework preamble barriers every engine until ~6.9us and the
        # sync engine serializes DMA descriptor-gen (~0.6us each) after
        # that; pre0's transfer completes ~11us. The profiled exec window
        # opens at the framework const-memsets (~5.9us) regardless, so the
        # warm-up occupies window that would otherwise be idle.
        warm = consts.tile([P, 256], bf16)
        nc.vector.memset(warm[:], 0.0)
        # NOTE: the warm-up group MUST own a dedicated PSUM bank — sharing a
        # pool slot with real accumulation groups hard-faults the device
        # (NRT_EXEC_UNIT_UNRECOVERABLE), reproduced twice.
        wps = psw.tile([P, 256], f32)
        NWARM = 18  # ends as the first real matmul's inputs land
        for i in range(NWARM):
            nc.tensor.matmul(wps[:], warm[:, :P], warm[:],
                             start=(i == 0), stop=(i == NWARM - 1))

        # Input DMAs all ride the sync-engine HW-DGE ring, which is FIFO in
        # emission order — so emit in exact consumption order with
        # geometrically ramped chunk sizes: small first chunks minimize the
        # time to the first matmul, large later chunks maximize stream
        # bandwidth (observed ~280 GB/s at 197KB vs ~430 GB/s at 786KB+).
        # NOTE (measured): the input path is supply-limited in the first
        # ~20us regardless — splitting pre0, finer w1 chunks, or spreading
        # DMAs across both HW-DGE rings all just move the stall around.
        pre0_sb = consts.tile([P, KO, TB + P], bf16)
        w1a_sb = consts.tile([P, SFP8, KO, P], bf16)
        w1b_sb = consts.tile([P, MH - SFP8, KO - 2, P], bf16)
        w1f8_sb = consts.tile([P, MH - SFP8, 2, P], f8e4)
        xe_sb = consts.tile([P, NBLK, KO, TB], bf16)
        b1_sb = consts.tile([P, MH], f32)
        b2_sb = consts.tile([P, KO], f32)
        w2_sb = consts.tile([P, KO, MH, P], bf16)

        # Descriptor generation is ~0.6-1.2us each, serialized on the sync
        # sequencer — keep the DMA count low. w1f8 is small (590KB) and
        # first needed at mh6 (~stream+7us), so one early DMA suffices.
        # mh=0 weights + xe blk0, split in three: splitting pipelines the
        # per-descriptor latency across queue slots, completing ~1.5us
        # earlier than a single DMA and accelerating the whole chain
        nc.sync.dma_start(pre0_sb[:, :2], pre0[:, :2])
        nc.sync.dma_start(pre0_sb[:, 2:4], pre0[:, 2:4])
        nc.sync.dma_start(pre0_sb[:, 4:], pre0[:, 4:])
        # b1 is tiny but a descriptor-gen slot costs ~0.6us of chain time;
        # mh1's weights are needed ~1us before the first activation needs
        # b1, so the weights go first.
        nc.sync.dma_start(w1a_sb[:, 1:2], w1a[:, 1:2])
        nc.sync.dma_start(b1_sb[:], b1)
        nc.sync.dma_start(w1a_sb[:, 2:SFP8], w1a[:, 2:SFP8])
        nc.sync.dma_start(w1f8_sb[:], w1f8)
        for lo, hi in ((SFP8, 8), (8, 12), (12, 18), (18, 24)):
            nc.sync.dma_start(w1b_sb[:, lo - SFP8:hi - SFP8],
                              w1b[:, lo - SFP8:hi - SFP8])
        nc.sync.dma_start(b2_sb[:], b2)
        for d in range(0, KO, 2):
            nc.sync.dma_start(w2_sb[:, d:d + 2], w2[:, d:d + 2])
        for blk in range(1, NBLK):
            nc.sync.dma_start(xe_sb[:, blk], xeT[:, blk])

        # --- Partial fp8: for mh >= SFP8, the ko 0-1 slice of mm1 runs as
        # one DoubleRow fp8 matmul (2 contraction planes per pass, ~235ns
        # vs 2x216ns bf16). Scale bookkeeping: host ships w1 as bf16*2048
        # (exact power-of-2) and w1f8 = e4m3(256*w1); device makes
        # xf8 = e4m3(8*x) on the idle vector engine, so fp8 products
        # (8x * 256w) and bf16 products (x * 2048w) share one PSUM scale,
        # removed exactly by scale=1/2048 in the gelu activation. 13.9% of
        # the total contraction in fp8 keeps max rel err ~1.88e-2 (<2e-2).
        xf8_sb = consts.tile([P, NBLK, 2, TB], f8e4)
        nc.vector.tensor_scalar_mul(xf8_sb[:, 0, 0], pre0_sb[:, 0, :TB], 8.0)
        nc.vector.tensor_scalar_mul(xf8_sb[:, 0, 1], pre0_sb[:, 1, :TB], 8.0)
        for blk in range(1, NBLK):
            nc.vector.tensor_scalar_mul(xf8_sb[:, blk], xe_sb[:, blk, 0:2], 8.0)

        for blk in range(NBLK):
            # mm1: hT[h, t] = gelu((sum_ko w1[ko,:].T @ x[ko,:]) / 2048 + b1)
            hT = hpool.tile([P, MH, TB], bf16)
            for mh in range(MH):
                ps = ps1.tile([P, TB], f32)
                if mh >= SFP8:
                    nc.tensor.matmul(ps[:], w1f8_sb[:, mh - SFP8],
                                     xf8_sb[:, blk], start=True, stop=False,
                                     perf_mode=DR)
                    ko_lo = 2
                else:
                    ko_lo = 0
                for ko in range(ko_lo, KO):
                    if mh == 0:
                        lhsT = pre0_sb[:, ko, TB:]
                    elif mh < SFP8:
                        lhsT = w1a_sb[:, mh, ko]
                    else:
                        lhsT = w1b_sb[:, mh - SFP8, ko - 2]
                    rhs = (pre0_sb[:, ko, :TB] if blk == 0
                           else xe_sb[:, blk, ko])
                    nc.tensor.matmul(ps[:], lhsT, rhs,
                                     start=(ko == 0), stop=(ko == KO - 1))
                nc.scalar.activation(hT[:, mh], ps[:], GELU,
                                     bias=b1_sb[:, mh:mh + 1],
                                     scale=1.0 / 2048.0)
            # mm2: yT[d, t] = sum_kh w2[kh,:].T @ hT[kh,:] + b2  (fp32 out)
            yT = ypool.tile([P, KO, TB], f32)
            for d in range(KO):
                if blk == NBLK - 1 and d == KO - 1:
                    # Tail: the very last psum group is split into two
                    # 256-column groups so the first half's activation and
                    # store overlap the second half's matmuls; only ~0.8us
                    # of activation+store remains after the last matmul.
                    for h0, h1 in ((0, TB // 2), (TB // 2, TB)):
                        ph = pst.tile([P, TB // 2], f32)
                        for kh in range(MH):
                            nc.tensor.matmul(ph[:], w2_sb[:, d, kh],
                                             hT[:, kh, h0:h1],
                                             start=(kh == 0),
                                             stop=(kh == MH - 1))
                        nc.scalar.activation(yT[:, d, h0:h1], ph[:],
                                             IDENT, bias=b2_sb[:, d:d + 1])
                        nc.sync.dma_start(out[:, blk, d, h0:h1],
                                          yT[:, d, h0:h1])
                    continue
                ps = ps2.tile([P, TB], f32)
                for kh in range(MH):
                    nc.tensor.matmul(ps[:], w2_sb[:, d, kh], hT[:, kh],
                                     start=(kh == 0), stop=(kh == MH - 1))
                nc.scalar.activation(yT[:, d], ps[:], IDENT,
                                     bias=b2_sb[:, d:d + 1])
                if blk == NBLK - 1:
                    # Last block: store per d-chunk so the tail stays fine-
                    # grained and the final transfer is small.
                    nc.sync.dma_start(out[:, blk, d], yT[:, d])
            if blk != NBLK - 1:
                # Mid-stream blocks: one batched 1.5MB store per block
                # (fewer descriptors for the sync sequencer and the final
                # queue drain; bandwidth is ample mid-stream).
                nc.sync.dma_start(out[:, blk], yT[:])

    # All DMAs ride the sync (SP) HW-DGE ring; the Pool SWDGE and
    # Activation HW-DGE rings are unused. Each declared queue costs ~3
    # semaphores that the codegen'd postamble resets one EVENT_SEMAPHORE at
    # a time (~115ns each on the Tensor sequencer) after the final barrier,
    # so shrink the unused rings to one queue each.
    try:
        for q in nc.m.queues:
            if q.name in ("qPoolDynamic", "qActDynamicHW"):
                q.num_queues = 1
    except Exception:
        pass

    nc.compile()
    return nc


def _route(x_flat, Wr):
    """Reproduce the reference router exactly: softmax -> top-2 -> renormalize
    -> capacity-limited keep in flat (token-major, k-inner) order."""
    logits = x_flat @ Wr
    m = logits.max(-1, keepdims=True)
    ex = np.exp(logits - m)
    probs = ex / ex.sum(-1, keepdims=True)
    n = np.arange(N_TOK)
    i1 = probs.argmax(-1)
    p1 = probs[n, i1]
    probs2 = probs.copy()
    probs2[n, i1] = -np.inf
    i2 = probs2.argmax(-1)
    p2 = probs[n, i2]
    s = p1 + p2
    e_flat = np.stack([i1, i2], -1).reshape(-1)          # [2N] expert ids
    p_flat = np.stack([p1 / s, p2 / s], -1).reshape(-1)  # [2N] combine weights
    order = np.argsort(e_flat, kind="stable")            # flat order per expert
    sorted_e = e_flat[order]
    starts = np.searchsorted(sorted_e, np.arange(E))
    ends = np.searchsorted(sorted_e, np.arange(E), side="right")
    toks, wgts = [], []
    for e in range(E):
        kept = order[starts[e] : min(ends[e], starts[e] + CAP)]
        toks.append(kept // TOPK)
        wgts.append(p_flat[kept].astype(np.float32))
    return toks, wgts


def kernel(x, Wr, w1, b1, w2, b2):
    _ensure_ntff_hook_importable()
    from concourse import bass_utils

    x = np.asarray(x, np.float32)
    Wr = np.asarray(Wr, np.float32)
    w1 = np.asarray(w1, np.float32)
    b1 = np.asarray(b1, np.float32)
    w2 = np.asarray(w2, np.float32)
    b2 = np.asarray(b2, np.float32)

    x_flat = x.reshape(N_TOK, D)
    toks, wgts = _route(x_flat, Wr)

    if "nc" not in _CACHE:
        _CACHE["nc"] = _build_nc()
    nc = _CACHE["nc"]

    bf = ml_dtypes.bfloat16
    in_maps = []
    for e in range(E):
        cnt = len(toks[e])
        xe = np.zeros((CAP, D), np.float32)
        xe[:cnt] = x_flat[toks[e]]
        xeT = np.ascontiguousarray(
            xe.reshape(NBLK, TB, KO, P).transpose(3, 0, 2, 1)).astype(bf)
        # w1 pre-scaled by 2^11 (exact in bf16) — see the PSUM scale note in
        # _build_nc; the gelu activation rescales by 1/2048. The fp8 mh
        # chunks (6..23) ship ko 0-1 as host-quantized e4m3 (scale 256,
        # single-rounded from fp32) and only ko 2-5 in bf16.
        SFP8 = 4
        w1r = np.ascontiguousarray(
            (w1[e] * 2048.0).reshape(KO, P, MH, P).transpose(1, 2, 0, 3)
        ).astype(bf)
        w1f8 = np.ascontiguousarray(
            np.clip(w1[e][:2 * P, SFP8 * P:] * 256.0, -240, 240)
            .reshape(2, P, MH - SFP8, P).transpose(1, 2, 0, 3)
        ).astype(ml_dtypes.float8_e4m3)
        in_maps.append({
            "pre0": np.ascontiguousarray(
                np.concatenate([xeT[:, 0], w1r[:, 0]], axis=-1)),
            "xeT": xeT,
            "w1a": np.ascontiguousarray(w1r[:, :SFP8]),
            "w1b": np.ascontiguousarray(w1r[:, SFP8:, 2:]),
            "w1f8": w1f8,
            "b1": np.ascontiguousarray(b1[e].reshape(MH, P).T),
            "w2": np.ascontiguousarray(
                w2[e].reshape(MH, P, KO, P).transpose(1, 2, 0, 3)).astype(bf),
            "b2": np.ascontiguousarray(b2[e].reshape(KO, P).T),
        })

    trace = bool(os.environ.get("MOE_TRACE"))
    try:
        res = bass_utils.run_bass_kernel_spmd(
            nc, in_maps, core_ids=list(range(NCORES)), trace=trace)
    except Exception:
        if trace or os.environ.get("BASS_TRACE"):
            # Profiling infrastructure failure — rerun without tracing.
            os.environ["BASS_NEVER_TRACE"] = "1"
            res = bass_utils.run_bass_kernel_spmd(
                nc, in_maps, core_ids=list(range(NCORES)), trace=False)
        else:
            raise
    _CACHE["last_results"] = res

    out = np.zeros((N_TOK, D), np.float32)
    for e in range(E):
        y = res.results[e]["out"]                      # [P, NBLK, KO, TB] f32
        y = y.transpose(1, 3, 2, 0).reshape(CAP, D)
        cnt = len(toks[e])
        # token ids are unique within one expert, so fancy-index += is safe
        out[toks[e]] += y[:cnt] * wgts[e][:, None]
    return out.reshape(B, S, D)



# revision 35
# speedup vs baseline: 1.0051x; 1.0051x over previous
"""Expert-parallel MoE kernel for Trainium2 (8 NeuronCores).

Strategy (hardcoded for B=4, S=2048, D=768, H=3072, E=8, K=2, cap_factor=1.5):
  - Host: router (x @ Wr, softmax, top-2, capacity-limited keep in flat order),
    then dispatch: gather each expert's kept tokens (<= capacity 1536) into a
    dense per-expert buffer. This is the "all-to-all dispatch" done at
    shard-time on the host.
  - Device (SPMD, one expert per core): dense fused FFN over the expert's
    token buffer: y = gelu(x @ w1 + b1) @ w2 + b2, bf16 matmuls with fp32
    accumulate, gelu/bias on the scalar engine. 13.9% of the mm1
    contraction (ko 0-1 for 20 of 24 h-chunks) runs as fp8 e4m3 DoubleRow
    matmuls (~235ns vs 2x216ns bf16, ~11us saved) — measured max rel err
    1.886e-2 vs the 2e-2 gate, numpy-predicted to 3 digits.
  - Host: combine: out[token] += combine_weight * y  (scatter-add, unshard).

Perf budget (exec window = first compute-engine slice -> last slice):
  ~6us framework preamble floor, ~7us warmup while pre0 streams (input DMA
  path caps ~260-280 GB/s early, 8-core HBM contention), ~166us dense
  matmul stream (<1us of gaps), ~11us tail (act+store+drain ~3us, codegen'd postamble
  that resets sems S[3..53] per engine ~8us — fixed, independent of queue
  declarations).

Self-contained: only needs numpy/ml_dtypes/concourse (+ axon jax devices).
"""
import os
import numpy as np
import ml_dtypes

B, S, D, H, E, TOPK = 4, 2048, 768, 3072, 8, 2
N_TOK = B * S
CAP = int((N_TOK / E) * 1.5)  # 1536
P = 128
TB = 512                      # token block (matmul free dim)
NBLK = CAP // TB              # 3
KO = D // P                   # 6  (d-chunks)
MH = H // P                   # 24 (h-chunks)
NCORES = 8

_CACHE = {}


def _ensure_ntff_hook_importable():
    """concourse.bass_utils' trace path does `from antenv.axon_hooks import
    get_axon_ntff_profile_hook`, which doesn't exist on slim axon images. If
    it's missing, register a stub so tracing degrades gracefully instead of
    crashing; when the axon .so with NRT-profile symbols is present, provide
    a working hook so NTFF profiling (HW exec time) works too."""
    import sys
    import types
    try:
        import antenv.axon_hooks  # noqa: F401
        return
    except ImportError:
        pass

    hook = None
    try:
        import contextlib
        import ctypes
        lib = ctypes.CDLL("/opt/axon/libaxon_pjrt.so")
        lib.axon_start_nrt_profile.argtypes = [
            ctypes.POINTER(ctypes.c_int64), ctypes.c_size_t]
        lib.axon_start_nrt_profile.restype = ctypes.c_int64
        lib.axon_stop_nrt_profile.argtypes = [ctypes.c_char_p]
        lib.axon_stop_nrt_profile.restype = ctypes.c_int64

        @contextlib.contextmanager
        def _hook(output_dir, device_ids):
            import jax
            jax.devices()
            if device_ids:
                ids = (ctypes.c_int64 * len(device_ids))(*device_ids)
                rc = lib.axon_start_nrt_profile(ids, len(device_ids))
            else:
                rc = lib.axon_start_nrt_profile(None, 0)
            if rc != 0:
                raise RuntimeError(f"axon_start_nrt_profile rc={rc}")
            try:
                yield
            finally:
                lib.axon_stop_nrt_profile(str(output_dir).encode())

        hook = _hook
    except Exception:
        hook = None

    mod = types.ModuleType("antenv.axon_hooks")
    mod.get_axon_ntff_profile_hook = lambda: hook
    mod.set_axon_ntff_profile_hook = lambda h: None
    sys.modules["antenv.axon_hooks"] = mod


def _build_nc():
    """Build + compile the per-core Bass program (identical on all 8 cores)."""
    from contextlib import ExitStack
    import concourse.mybir as mybir
    import concourse.tile as tile
    from concourse import bacc

    nc = bacc.Bacc("TRN2", target_bir_lowering=False, debug=False,
                   num_devices=NCORES)
    f32, bf16 = mybir.dt.float32, mybir.dt.bfloat16

    # Layouts (host pre-arranged so every DMA is contiguous):
    #  xeT[pi, blk, ko, t]  = x_e[blk*TB + t, ko*P + pi]
    #  w1 [pi, mh, ko, hi]  = w1_e[ko*P + pi, mh*P + hi]   (lhsT tiles for mm1)
    #  w2 [pi, d,  kh, di]  = w2_e[kh*P + pi, d*P + di]    (lhsT tiles for mm2)
    #  b1 [pi, mh] = b1_e[mh*P + pi],  b2[pi, d] = b2_e[d*P + pi]
    #  out[pi, blk, ko, t]  = y_e[blk*TB + t, ko*P + pi]
    #  pre0[pi, ko, 0:TB] = xe blk0; [TB:TB+P] = w1_e[ko*P+pi, 0:P] (mh=0 tiles)
    GELU = mybir.ActivationFunctionType.Gelu
    IDENT = mybir.ActivationFunctionType.Identity
    DR = mybir.MatmulPerfMode.DoubleRow
    f8e4 = mybir.dt.float8e4
    SFP8 = MH - 20  # mh >= SFP8 run ko 0-1 as one fp8 DoubleRow matmul

    pre0 = nc.dram_tensor("pre0", [P, KO, TB + P], bf16, kind="ExternalInput").ap()
    xeT = nc.dram_tensor("xeT", [P, NBLK, KO, TB], bf16, kind="ExternalInput").ap()
    # w1 ships in three pieces: full bf16 for the non-fp8 mh chunks, ko 2-5
    # bf16 for the fp8 ones, and the ko 0-1 slice as host-quantized e4m3 —
    # 590KB less on the supply-critical early DMA path than shipping full
    # bf16 + converting on device, and the w leg is single-rounded
    # (fp32->e4m3), which also improves max rel err 1.90e-2 -> 1.79e-2.
    w1a = nc.dram_tensor("w1a", [P, SFP8, KO, P], bf16, kind="ExternalInput").ap()
    w1b = nc.dram_tensor("w1b", [P, MH - SFP8, KO - 2, P], bf16,
                         kind="ExternalInput").ap()
    w1f8 = nc.dram_tensor("w1f8", [P, MH - SFP8, 2, P], f8e4,
                          kind="ExternalInput").ap()
    b1 = nc.dram_tensor("b1", [P, MH], f32, kind="ExternalInput").ap()
    w2 = nc.dram_tensor("w2", [P, KO, MH, P], bf16, kind="ExternalInput").ap()
    b2 = nc.dram_tensor("b2", [P, KO], f32, kind="ExternalInput").ap()
    out = nc.dram_tensor("out", [P, NBLK, KO, TB], f32, kind="ExternalOutput").ap()

    with tile.TileContext(nc) as tc, ExitStack() as ctx:
        consts = ctx.enter_context(tc.tile_pool(name="consts", bufs=1))
        hpool = ctx.enter_context(tc.tile_pool(name="hpool", bufs=2))
        ypool = ctx.enter_context(tc.tile_pool(name="ypool", bufs=2))
        ps1 = ctx.enter_context(tc.tile_pool(name="ps1", bufs=3, space="PSUM"))
        ps2 = ctx.enter_context(tc.tile_pool(name="ps2", bufs=2, space="PSUM"))
        pst = ctx.enter_context(tc.tile_pool(name="pst", bufs=2, space="PSUM"))
        psw = ctx.enter_context(tc.tile_pool(name="psw", bufs=1, space="PSUM"))

        # PE warm-up: cold matmuls ramp the PE clock 1.2->2.4 GHz while the
        # first input DMAs are in flight, so the real stream runs hot from
        # its first instruction. Nothing can start before ~7.2us: the
        # framework preamble barriers every engine until ~6.9us and the
        # sync engine serializes DMA descriptor-gen (~0.6us each) after
        # that; pre0's transfer completes ~11us. The profiled exec window
        # opens at the framework const-memsets (~5.9us) regardless, so the
        # warm-up occupies window that would otherwise be idle.
        warm = consts.tile([P, 256], bf16)
        nc.vector.memset(warm[:], 0.0)
        # NOTE: the warm-up group MUST own a dedicated PSUM bank — sharing a
        # pool slot with real accumulation groups hard-faults the device
        # (NRT_EXEC_UNIT_UNRECOVERABLE), reproduced twice.
        wps = psw.tile([P, 256], f32)
        NWARM = 18  # ends as the first real matmul's inputs land
        for i in range(NWARM):
            nc.tensor.matmul(wps[:], warm[:, :P], warm[:],
                             start=(i == 0), stop=(i == NWARM - 1))

        # Input DMAs all ride the sync-engine HW-DGE ring, which is FIFO in
        # emission order — so emit in exact consumption order with
        # geometrically ramped chunk sizes: small first chunks minimize the
        # time to the first matmul, large later chunks maximize stream
        # bandwidth (observed ~280 GB/s at 197KB vs ~430 GB/s at 786KB+).
        # NOTE (measured): the input path is supply-limited in the first
        # ~20us regardless — splitting pre0, finer w1 chunks, or spreading
        # DMAs across both HW-DGE rings all just move the stall around.
        pre0_sb = consts.tile([P, KO, TB + P], bf16)
        w1a_sb = consts.tile([P, SFP8, KO, P], bf16)
        w1b_sb = consts.tile([P, MH - SFP8, KO - 2, P], bf16)
        w1f8_sb = consts.tile([P, MH - SFP8, 2, P], f8e4)
        xe_sb = consts.tile([P, NBLK, KO, TB], bf16)
        b1_sb = consts.tile([P, MH], f32)
        b2_sb = consts.tile([P, KO], f32)
        w2_sb = consts.tile([P, KO, MH, P], bf16)

        # Descriptor generation is ~0.6-1.2us each, serialized on the sync
        # sequencer — keep the DMA count low. w1f8 is small (590KB) and
        # first needed at mh6 (~stream+7us), so one early DMA suffices.
        # mh=0 weights + xe blk0, split in two: pipelines the
        # per-descriptor latency across queue slots, completing ~1.2us
        # earlier than a single DMA and accelerating the whole chain
        # (a 3-way split is worse: the extra ~0.7us descriptor-gen slot
        # delays everything behind it in the ring)
        nc.sync.dma_start(pre0_sb[:, :3], pre0[:, :3])
        nc.sync.dma_start(pre0_sb[:, 3:], pre0[:, 3:])
        # b1 is tiny but a descriptor-gen slot costs ~0.6us of chain time;
        # mh1's weights are needed ~1us before the first activation needs
        # b1, so the weights go first.
        nc.sync.dma_start(w1a_sb[:, 1:2], w1a[:, 1:2])
        nc.sync.dma_start(b1_sb[:], b1)
        nc.sync.dma_start(w1a_sb[:, 2:SFP8], w1a[:, 2:SFP8])
        nc.sync.dma_start(w1f8_sb[:], w1f8)
        for lo, hi in ((SFP8, 8), (8, 12), (12, 18), (18, 24)):
            nc.sync.dma_start(w1b_sb[:, lo - SFP8:hi - SFP8],
                              w1b[:, lo - SFP8:hi - SFP8])
        nc.sync.dma_start(b2_sb[:], b2)
        for d in range(0, KO, 2):
            nc.sync.dma_start(w2_sb[:, d:d + 2], w2[:, d:d + 2])
        for blk in range(1, NBLK):
            nc.sync.dma_start(xe_sb[:, blk], xeT[:, blk])

        # --- Partial fp8: for mh >= SFP8, the ko 0-1 slice of mm1 runs as
        # one DoubleRow fp8 matmul (2 contraction planes per pass, ~235ns
        # vs 2x216ns bf16). Scale bookkeeping: host ships w1 as bf16*2048
        # (exact power-of-2) and w1f8 = e4m3(256*w1); device makes
        # xf8 = e4m3(8*x) on the idle vector engine, so fp8 products
        # (8x * 256w) and bf16 products (x * 2048w) share one PSUM scale,
        # removed exactly by scale=1/2048 in the gelu activation. 13.9% of
        # the total contraction in fp8 keeps max rel err ~1.88e-2 (<2e-2).
        xf8_sb = consts.tile([P, NBLK, 2, TB], f8e4)
        nc.vector.tensor_scalar_mul(xf8_sb[:, 0, 0], pre0_sb[:, 0, :TB], 8.0)
        nc.vector.tensor_scalar_mul(xf8_sb[:, 0, 1], pre0_sb[:, 1, :TB], 8.0)
        for blk in range(1, NBLK):
            nc.vector.tensor_scalar_mul(xf8_sb[:, blk], xe_sb[:, blk, 0:2], 8.0)

        for blk in range(NBLK):
            # mm1: hT[h, t] = gelu((sum_ko w1[ko,:].T @ x[ko,:]) / 2048 + b1)
            hT = hpool.tile([P, MH, TB], bf16)
            for mh in range(MH):
                ps = ps1.tile([P, TB], f32)
                if mh >= SFP8:
                    nc.tensor.matmul(ps[:], w1f8_sb[:, mh - SFP8],
                                     xf8_sb[:, blk], start=True, stop=False,
                                     perf_mode=DR)
                    ko_lo = 2
                else:
                    ko_lo = 0
                for ko in range(ko_lo, KO):
                    if mh == 0:
                        lhsT = pre0_sb[:, ko, TB:]
                    elif mh < SFP8:
                        lhsT = w1a_sb[:, mh, ko]
                    else:
                        lhsT = w1b_sb[:, mh - SFP8, ko - 2]
                    rhs = (pre0_sb[:, ko, :TB] if blk == 0
                           else xe_sb[:, blk, ko])
                    nc.tensor.matmul(ps[:], lhsT, rhs,
                                     start=(ko == 0), stop=(ko == KO - 1))
                nc.scalar.activation(hT[:, mh], ps[:], GELU,
                                     bias=b1_sb[:, mh:mh + 1],
                                     scale=1.0 / 2048.0)
            # mm2: yT[d, t] = sum_kh w2[kh,:].T @ hT[kh,:] + b2  (fp32 out)
            yT = ypool.tile([P, KO, TB], f32)
            for d in range(KO):
                if blk == NBLK - 1 and d == KO - 1:
                    # Tail: the very last psum group is split into two
                    # 256-column groups so the first half's activation and
                    # store overlap the second half's matmuls; only ~0.8us
                    # of activation+store remains after the last matmul.
                    for h0, h1 in ((0, TB // 2), (TB // 2, TB)):
                        ph = pst.tile([P, TB // 2], f32)
                        for kh in range(MH):
                            nc.tensor.matmul(ph[:], w2_sb[:, d, kh],
                                             hT[:, kh, h0:h1],
                                             start=(kh == 0),
                                             stop=(kh == MH - 1))
                        nc.scalar.activation(yT[:, d, h0:h1], ph[:],
                                             IDENT, bias=b2_sb[:, d:d + 1])
                        nc.sync.dma_start(out[:, blk, d, h0:h1],
                                          yT[:, d, h0:h1])
                    continue
                ps = ps2.tile([P, TB], f32)
                for kh in range(MH):
                    nc.tensor.matmul(ps[:], w2_sb[:, d, kh], hT[:, kh],
                                     start=(kh == 0), stop=(kh == MH - 1))
                nc.scalar.activation(yT[:, d], ps[:], IDENT,
                                     bias=b2_sb[:, d:d + 1])
                if blk == NBLK - 1:
                    # Last block: store per d-chunk so the tail stays fine-
                    # grained and the final transfer is small.
                    nc.sync.dma_start(out[:, blk, d], yT[:, d])
            if blk != NBLK - 1:
                # Mid-stream blocks: one batched 1.5MB store per block
                # (fewer descriptors for the sync sequencer and the final
                # queue drain; bandwidth is ample mid-stream).
                nc.sync.dma_start(out[:, blk], yT[:])

    # All DMAs ride the sync (SP) HW-DGE ring; the Pool SWDGE and
    # Activation HW-DGE rings are unused. Each declared queue costs ~3
    # semaphores that the codegen'd postamble resets one EVENT_SEMAPHORE at
    # a time (~115ns each on the Tensor sequencer) after the final barrier,
    # so shrink the unused rings to one queue each.
    try:
        for q in nc.m.queues:
            if q.name in ("qPoolDynamic", "qActDynamicHW"):
                q.num_queues = 1
    except Exception:
        pass

    nc.compile()
    return nc


def _route(x_flat, Wr):
    """Reproduce the reference router exactly: softmax -> top-2 -> renormalize
    -> capacity-limited keep in flat (token-major, k-inner) order."""
    logits = x_flat @ Wr
    m = logits.max(-1, keepdims=True)
    ex = np.exp(logits - m)
    probs = ex / ex.sum(-1, keepdims=True)
    n = np.arange(N_TOK)
    i1 = probs.argmax(-1)
    p1 = probs[n, i1]
    probs2 = probs.copy()
    probs2[n, i1] = -np.inf
    i2 = probs2.argmax(-1)
    p2 = probs[n, i2]
    s = p1 + p2
    e_flat = np.stack([i1, i2], -1).reshape(-1)          # [2N] expert ids
    p_flat = np.stack([p1 / s, p2 / s], -1).reshape(-1)  # [2N] combine weights
    order = np.argsort(e_flat, kind="stable")            # flat order per expert
    sorted_e = e_flat[order]
    starts = np.searchsorted(sorted_e, np.arange(E))
    ends = np.searchsorted(sorted_e, np.arange(E), side="right")
    toks, wgts = [], []
    for e in range(E):
        kept = order[starts[e] : min(ends[e], starts[e] + CAP)]
        toks.append(kept // TOPK)
        wgts.append(p_flat[kept].astype(np.float32))
    return toks, wgts


def kernel(x, Wr, w1, b1, w2, b2):
    _ensure_ntff_hook_importable()
    from concourse import bass_utils

    x = np.asarray(x, np.float32)
    Wr = np.asarray(Wr, np.float32)
    w1 = np.asarray(w1, np.float32)
    b1 = np.asarray(b1, np.float32)
    w2 = np.asarray(w2, np.float32)
    b2 = np.asarray(b2, np.float32)

    x_flat = x.reshape(N_TOK, D)
    toks, wgts = _route(x_flat, Wr)

    if "nc" not in _CACHE:
        _CACHE["nc"] = _build_nc()
    nc = _CACHE["nc"]

    bf = ml_dtypes.bfloat16
    in_maps = []
    for e in range(E):
        cnt = len(toks[e])
        xe = np.zeros((CAP, D), np.float32)
        xe[:cnt] = x_flat[toks[e]]
        xeT = np.ascontiguousarray(
            xe.reshape(NBLK, TB, KO, P).transpose(3, 0, 2, 1)).astype(bf)
        # w1 pre-scaled by 2^11 (exact in bf16) — see the PSUM scale note in
        # _build_nc; the gelu activation rescales by 1/2048. The fp8 mh
        # chunks (6..23) ship ko 0-1 as host-quantized e4m3 (scale 256,
        # single-rounded from fp32) and only ko 2-5 in bf16.
        SFP8 = 4
        w1r = np.ascontiguousarray(
            (w1[e] * 2048.0).reshape(KO, P, MH, P).transpose(1, 2, 0, 3)
        ).astype(bf)
        w1f8 = np.ascontiguousarray(
            np.clip(w1[e][:2 * P, SFP8 * P:] * 256.0, -240, 240)
            .reshape(2, P, MH - SFP8, P).transpose(1, 2, 0, 3)
        ).astype(ml_dtypes.float8_e4m3)
        in_maps.append({
            "pre0": np.ascontiguousarray(
                np.concatenate([xeT[:, 0], w1r[:, 0]], axis=-1)),
            "xeT": xeT,
            "w1a": np.ascontiguousarray(w1r[:, :SFP8]),
            "w1b": np.ascontiguousarray(w1r[:, SFP8:, 2:]),
            "w1f8": w1f8,
            "b1": np.ascontiguousarray(b1[e].reshape(MH, P).T),
            "w2": np.ascontiguousarray(
                w2[e].reshape(MH, P, KO, P).transpose(1, 2, 0, 3)).astype(bf),
            "b2": np.ascontiguousarray(b2[e].reshape(KO, P).T),
        })

    trace = bool(os.environ.get("MOE_TRACE"))
    try:
        res = bass_utils.run_bass_kernel_spmd(
            nc, in_maps, core_ids=list(range(NCORES)), trace=trace)
    except Exception:
        if trace or os.environ.get("BASS_TRACE"):
            # Profiling infrastructure failure — rerun without tracing.
            os.environ["BASS_NEVER_TRACE"] = "1"
            res = bass_utils.run_bass_kernel_spmd(
                nc, in_maps, core_ids=list(range(NCORES)), trace=False)
        else:
            raise
    _CACHE["last_results"] = res

    out = np.zeros((N_TOK, D), np.float32)
    for e in range(E):
        y = res.results[e]["out"]                      # [P, NBLK, KO, TB] f32
        y = y.transpose(1, 3, 2, 0).reshape(CAP, D)
        cnt = len(toks[e])
        # token ids are unique within one expert, so fancy-index += is safe
        out[toks[e]] += y[:cnt] * wgts[e][:, None]
    return out.reshape(B, S, D)

